# revision 1
# baseline (speedup 1.0000x reference)
"""Trainium2 Bass kernel for nn_CrossAttentionBlock.

Reference computation (B=16384, C=1024, D=128):
    g_x     = x0 @ g_w.T + g_b          # [B, D]
    theta_x = x1 @ theta_w.T + theta_b  # [B, D]
    phi_x   = x1 @ phi_w.T + phi_b      # [B, D]
    f[b,i,j] = phi_x[b,i] * theta_x[b,j]
    attn = softmax(f, axis=-1)
    y[b,i] = sum_j attn[b,i,j] * g_x[b,j]
    out = y @ W_w.T + W_b + x0          # [B, C]

Unnormalized form used on-chip (no max-subtraction needed: |f| <= ~40, exp
fits fp32/bf16 comfortably):
    E_T[j,i] = exp(theta[b,j] * phi[b,i])        (per b, j on partitions)
    num[i] = sum_j g[b,j] * E_T[j,i]   den[i] = sum_j E_T[j,i]
    y[b,i] = num[i] / den[i]

Sharding: pure data parallel over batch across 8 cores (2048 rows/core).

Per-core pipeline:
  P1: theta/phi projections -> [b,d] fp16 tiles; g projection -> g_xT [d,b]
      interleaved with ones into g1 [d, 2b] bf16.
  P2: per-b rank-1 outer-product matmuls (K=1, lhsT=theta-row, rhs=phi-row,
      4-way row-tiled at partitions {0,32,64,96} via a realignment DMA) write
      f_T [j,i] into PSUM; grouped ACT exp (PSUM->SBUF, bf16) produces E_T;
      per-b reduce matmuls (lhsT=E_T_b, rhs=[g|1]) accumulate num/den in
      PSUM; DVE reciprocal+mul produce y_T [d,b] bf16.
  P3: final matmul (lhsT=y_T group, rhs=W_w.T, N=1024) + residual add + DMA.
"""

import os
from contextlib import ExitStack

import numpy as np

import concourse.bass as bass
import concourse.tile as tile
from concourse import bacc
from concourse import mybir

F32 = mybir.dt.float32
F16 = mybir.dt.float16
BF16 = mybir.dt.bfloat16

NCORES = 8
B, C, D = 16384, 1024, 128
KC = C // 128  # 8 contraction chunks for the projections

# batch rows per f/E tile in the attention phase (1024 fp32 = 2 PSUM banks)
FTILE = 8


def build_bass(bc: int, reps: int = 1):
    """Build the per-core bass program for a batch slice of `bc` rows.

    reps>1 repeats the whole pipeline (for (T_R - T_1)/(R-1) timing)."""
    ng = bc // 128  # groups of 128 rows
    nq = max(1, bc // 512)  # 512-row groups for the g projection
    qsz = min(bc, 512)
    n_ftiles = (bc + FTILE - 1) // FTILE

    nc = bacc.Bacc(trn_type="TRN2")

    x1t = nc.dram_tensor("x1t", [C, bc], F16, kind="ExternalInput")
    x0t = nc.dram_tensor("x0t", [C, bc], F16, kind="ExternalInput")
    x0r = nc.dram_tensor("x0r", [bc, C], F16, kind="ExternalInput")
    wc = nc.dram_tensor("wc", [C, 2 * D], F16, kind="ExternalInput")
    gwt = nc.dram_tensor("gwt", [C, D], F16, kind="ExternalInput")
    wwt = nc.dram_tensor("wwt", [D, C], BF16, kind="ExternalInput")
    btp = nc.dram_tensor("btp", [128, 2 * D], F32, kind="ExternalInput")
    gb = nc.dram_tensor("gb", [D, 1], F32, kind="ExternalInput")
    out = nc.dram_tensor("out", [bc, C], F32, kind="ExternalOutput")

    with tile.TileContext(nc) as tc, ExitStack() as ctx:
        singles = ctx.enter_context(tc.tile_pool(name="singles", bufs=1))

        # ---- static weights / constants in SBUF ----
        wc_sb = singles.tile([128, KC, 2 * D], F16)  # [c-part, chunk, 256]
        nc.sync.dma_start(wc_sb, wc[:, :].rearrange("(k p) d -> p k d", p=128))
        gwt_sb = singles.tile([128, KC, D], F16)
        nc.sync.dma_start(gwt_sb, gwt[:, :].rearrange("(k p) d -> p k d", p=128))
        wwt_sb = singles.tile([128, C], BF16)
        nc.sync.dma_start(wwt_sb, wwt[:, :])
        btp_sb = singles.tile([128, 2 * D], F32)
        nc.sync.dma_start(btp_sb, btp[:, :])
        gb_sb = singles.tile([128, 1], F32)
        nc.sync.dma_start(gb_sb, gb[:, :])

        # persistent per-core activations
        tp16 = singles.tile([128, ng, 2 * D], F16)  # [theta|phi] fp16, [b-part, G, 256]
        g1 = singles.tile([128, 2 * bc], BF16)  # g_xT interleaved with ones [d, 2b]
        y16 = singles.tile([128, bc], BF16)  # y_T [d, b] bf16
        nc.vector.memset(g1, 1.0)

        from contextlib import nullcontext
        rep_ctx = tc.For_i(0, reps, 1) if reps > 1 else nullcontext()
        with rep_ctx:
            # ===== P1 interleaved into P2: per-group projections =====
            with (
                tc.tile_pool(name="xin", bufs=3) as xin,
                tc.tile_pool(name="xg", bufs=2) as xg,
                tc.tile_pool(name="projpsum", bufs=1, space="PSUM") as projpsum,
                tc.tile_pool(name="fpsum", bufs=2, space="PSUM") as fpsum,
                tc.tile_pool(name="ndpsum", bufs=1, space="PSUM") as ndpsum,
                tc.tile_pool(name="opsum", bufs=2, space="PSUM") as opsum,
                tc.tile_pool(name="epool", bufs=3) as epool,
                tc.tile_pool(name="ndsb", bufs=2) as ndsb,
                tc.tile_pool(name="rec", bufs=2) as rec,
                tc.tile_pool(name="resid", bufs=6) as resid,
                tc.tile_pool(name="osb", bufs=3) as osb,
            ):
                g1v = g1.rearrange("p (b two) -> p b two", two=2)

                x1_tiles = [None] * ng
                x0_tiles = [None] * nq

                def emit_x1_dma(G):
                    x1_tiles[G] = xin.tile([128, KC, 128], F16, tag="xin", name="xint")
                    nc.sync.dma_start(
                        x1_tiles[G],
                        x1t[:, G * 128 : (G + 1) * 128].rearrange(
                            "(k p) b -> p k b", p=128
                        ),
                    )

                def emit_x0_dma(q):
                    x0_tiles[q] = xg.tile([128, KC, qsz], F16, tag="xg", name="xgt")
                    nc.sync.dma_start(
                        x0_tiles[q],
                        x0t[:, q * qsz : (q + 1) * qsz].rearrange(
                            "(k p) b -> p k b", p=128
                        ),
                    )

                def emit_proj_tp(G):
                    # theta/phi projection for one 128-row group
                    pt = projpsum.tile([128, 512], F32, tag="pp", name="ppt")
                    xt = x1_tiles[G]
                    for k in range(KC):
                        nc.tensor.matmul(
                            pt[:, : 2 * D], lhsT=xt[:, k, :], rhs=wc_sb[:, k, :],
                            start=(k == 0), stop=(k == KC - 1),
                        )
                    nc.vector.tensor_add(tp16[:, G, :], pt[:, : 2 * D], btp_sb)

                def emit_proj_g(q):
                    gp = projpsum.tile([128, 512], F32, tag="pp", name="gpt")
                    gp = gp[:, :qsz]
                    xt = x0_tiles[q]
                    for k in range(KC):
                        nc.tensor.matmul(
                            gp, lhsT=gwt_sb[:, k, :], rhs=xt[:, k, :],
                            start=(k == 0), stop=(k == KC - 1),
                        )
                    nc.vector.tensor_scalar_add(
                        g1v[:, q * qsz : (q + 1) * qsz, 0], gp, gb_sb
                    )

                # slot s -> (group G, b-local): natural order. Theta/phi rows
                # are realigned onto partitions 0-3 (b mod 4 -> partition), with
                # phi embedded block-diagonally in a zero-padded buffer, so one
                # K=4 matmul at tile_position (0,0) computes 4 outer products.
                # (Concurrent row-tiled positions crash the exec unit on this HW.)
                QB = 4  # batch rows per outer matmul
                QROWS = 64  # b-rows per realigned buffer quarter
                nquart = (bc + QROWS - 1) // QROWS
                QT = QROWS // QB  # quads per quarter

                f_tiles = [None] * n_ftiles
                e_tiles = [None] * n_ftiles
                nd_tiles = [None] * ng
                xr_tiles = [None] * ng

                gpq = max(1, qsz // 128)  # groups per g-projection block

                # ping-pong persistent realign buffers (zeros are static)
                thbuf = [
                    singles.tile([128, QT * D], F16, name=f"thbuf{i}")
                    for i in range(2)
                ]
                phbuf = [
                    singles.tile([128, QT * QB * D], F16, name=f"phbuf{i}")
                    for i in range(2)
                ]
                for i in range(2):
                    nc.vector.memset(phbuf[i], 0.0)

                def emit_realign(q):
                    # rows [q*QROWS, (q+1)*QROWS): theta row (QB*t+p) to
                    # (partition p, offset t*128); phi row to (partition p,
                    # offset t*512 + p*128) inside the zeroed buffer.
                    Gq, blq = divmod(q * QROWS, 128)
                    tb, pb = thbuf[q % 2], phbuf[q % 2]
                    for p in range(QB):
                        src_t = tp16[:, Gq, 0:D][blq + p : blq + QROWS : QB, :]
                        dst_t = tb[p : p + 1, :].rearrange("o (t e) -> o t e", e=D)
                        nc.gpsimd.dma_start(dst_t, src_t)
                        src_p = tp16[:, Gq, D : 2 * D][blq + p : blq + QROWS : QB, :]
                        dst_p = pb[p : p + 1, :].rearrange(
                            "o (t f) -> o t f", f=QB * D
                        )[:, :, p * D : (p + 1) * D]
                        nc.gpsimd.dma_start(dst_p, src_p)

                def emit_outers(T):
                    lo, hi = T * FTILE, min((T + 1) * FTILE, bc)
                    f_tiles[T] = fpsum.tile([128, FTILE * 128], F32, tag="f", name="ftile")
                    for s in range(lo, hi, QB):
                        G, bl = divmod(s, 128)
                        if bl == 0:
                            # prefetch input DMAs and run projections one
                            # group ahead so the PE never stalls on loads.
                            if G == 0:
                                for Gp in range(min(3, ng)):
                                    emit_x1_dma(Gp)
                                emit_x0_dma(0)
                                for Gp in range(min(2, ng)):
                                    emit_proj_tp(Gp)
                            else:
                                if G + 2 < ng:
                                    emit_x1_dma(G + 2)
                                if G + 1 < ng:
                                    emit_proj_tp(G + 1)
                            if (G + 1) % gpq == 0 and (G + 1) // gpq < nq:
                                emit_x0_dma((G + 1) // gpq)
                            if G % gpq == 0 and G > 0:
                                emit_proj_g(G // gpq)
                            nd_tiles[G] = ndpsum.tile([128, 2 * 128], F32, tag="nd", name="ndt")
                            xr_tiles[G] = resid.tile([128, C], F16, tag="xr", name="xrt")
                            nc.sync.dma_start(
                                xr_tiles[G], x0r[G * 128 : (G + 1) * 128, :]
                            )
                        q, r = divmod(s, QROWS)
                        if r == 0:
                            if q == 0:
                                emit_realign(0)
                            if q + 1 < nquart:
                                emit_realign(q + 1)
                        t = r // QB  # quad index within quarter
                        j = s - lo
                        nc.tensor.matmul(
                            f_tiles[T][:, j * 128 : (j + QB) * 128],
                            lhsT=thbuf[q % 2][0:QB, t * D : (t + 1) * D],
                            rhs=phbuf[q % 2][0:QB, t * QB * D : (t + 1) * QB * D],
                        )

                def emit_exp(T):
                    lo, hi = T * FTILE, min((T + 1) * FTILE, bc)
                    n = hi - lo
                    e_tiles[T] = epool.tile([128, FTILE * 128], BF16, tag="e", name="etile")
                    nc.scalar.activation(
                        e_tiles[T][:, : n * 128],
                        f_tiles[T][:, : n * 128],
                        mybir.ActivationFunctionType.Exp,
                    )
                    if os.environ.get("K_DOUBLE_EXP"):
                        e2 = epool.tile([128, FTILE * 128], BF16, tag="e2", name="e2tile")
                        nc.scalar.activation(
                            e2[:, : n * 128],
                            f_tiles[T][:, : n * 128],
                            mybir.ActivationFunctionType.Exp,
                        )

                def emit_reduces(T):
                    lo, hi = T * FTILE, min((T + 1) * FTILE, bc)
                    nrep = 2 if os.environ.get("K_DOUBLE_REDUCE") else 1
                    for s in range(lo, hi):
                        G, bl = divmod(s, 128)
                        j = s - lo
                        for _ in range(nrep):
                            nc.tensor.matmul(
                                nd_tiles[G][:, 2 * bl : 2 * bl + 2],
                                lhsT=e_tiles[T][:, j * 128 : (j + 1) * 128],
                                rhs=g1[:, 2 * (G * 128 + bl) : 2 * (G * 128 + bl) + 2],
                            )

                def emit_final(G):
                    ot = osb.tile([128, C], F32, tag="ot", name="ott")
                    for h in range(2):
                        op = opsum.tile([128, 512], F32, tag="op", name="opt")
                        nc.tensor.matmul(
                            op,
                            lhsT=y16[:, G * 128 : (G + 1) * 128],
                            rhs=wwt_sb[:, h * 512 : (h + 1) * 512],
                        )
                        nc.vector.tensor_add(
                            ot[:, h * 512 : (h + 1) * 512],
                            op,
                            xr_tiles[G][:, h * 512 : (h + 1) * 512],
                        )
                    nc.sync.dma_start(out[G * 128 : (G + 1) * 128, :], ot)

                def emit_divide(G):
                    nd = ndsb.tile([128, 256], F32, tag="ndsb")
                    nc.vector.tensor_copy(nd, nd_tiles[G])
                    ndv = nd.rearrange("p (b two) -> p b two", two=2)
                    r = rec.tile([128, 128], F32, tag="rec")
                    nc.vector.reciprocal(r, ndv[:, :, 1])
                    nc.vector.tensor_mul(
                        y16[:, G * 128 : (G + 1) * 128], ndv[:, :, 0], r
                    )

                # software-pipelined emission: outers(T), exp(T-1), reduces(T-2)
                for T in range(n_ftiles + 2):
                    if T < n_ftiles:
                        emit_outers(T)
                    if T == 1:
                        emit_proj_g(0)  # x0 DMA was issued at T=0; MMs here
                    if 1 <= T <= n_ftiles:
                        emit_exp(T - 1)
                    if T >= 2:
                        Tr = T - 2
                        emit_reduces(Tr)
                        # divide+final for any group fully reduced by tile Tr
                        hi = min((Tr + 1) * FTILE, bc)
                        lo = Tr * FTILE
                        for G in range(lo // 128, (hi + 127) // 128):
                            if lo < (G + 1) * 128 <= hi:
                                emit_divide(G)
                                emit_final(G)
                        if hi == bc and bc % 128 != 0:
                            emit_divide(bc // 128)
                            emit_final(bc // 128)

    nc.compile()
    return nc


_BASS_CACHE = {}


def _get_bass(bc):
    if bc not in _BASS_CACHE:
        _BASS_CACHE[bc] = build_bass(bc)
    return _BASS_CACHE[bc]


def make_core_inputs(x0, x1, g_w, g_b, theta_w, theta_b, phi_w, phi_b, W_w, W_b,
                     bc=None, ncores=NCORES):
    """Host-side preprocessing -> list of per-core input dicts."""
    n = x0.shape[0] if bc is None else bc * ncores
    bc = n // ncores

    x0 = np.asarray(x0, np.float32)[:n]
    x1 = np.asarray(x1, np.float32)[:n]
    x1t = np.ascontiguousarray(x1.T.astype(np.float16))
    x0t = np.ascontiguousarray(x0.T.astype(np.float16))
    x0r = x0 if not np.any(W_b) else (x0 + np.asarray(W_b, np.float32)[None, :])
    x0r = np.ascontiguousarray(x0r, dtype=np.float16)

    wc = np.ascontiguousarray(
        np.concatenate([np.asarray(theta_w).T, np.asarray(phi_w).T], axis=1).astype(np.float16)
    )  # [C, 2D]
    gwt = np.ascontiguousarray(np.asarray(g_w).T.astype(np.float16))  # [C, D]
    import ml_dtypes
    wwt = np.ascontiguousarray(np.asarray(W_w).T.astype(ml_dtypes.bfloat16))  # [D, C]
    btp = np.ascontiguousarray(
        np.tile(np.concatenate([np.asarray(theta_b), np.asarray(phi_b)])[None, :], (128, 1)).astype(np.float32)
    )
    gbc = np.ascontiguousarray(np.asarray(g_b, np.float32).reshape(D, 1))

    in_maps = []
    for c in range(ncores):
        sl = slice(c * bc, (c + 1) * bc)
        in_maps.append(
            {
                "x1t": np.ascontiguousarray(x1t[:, sl]),
                "x0t": np.ascontiguousarray(x0t[:, sl]),
                "x0r": np.ascontiguousarray(x0r[sl]),
                "wc": wc,
                "gwt": gwt,
                "wwt": wwt,
                "btp": btp,
                "gb": gbc,
            }
        )
    return in_maps, bc


def kernel(x0, x1, g_w, g_b, theta_w, theta_b, phi_w, phi_b, W_w, W_b):
    from concourse.bass_utils import run_bass_kernel_spmd

    in_maps, bc = make_core_inputs(
        x0, x1, g_w, g_b, theta_w, theta_b, phi_w, phi_b, W_w, W_b
    )
    nc = _get_bass(bc)
    res = run_bass_kernel_spmd(nc, in_maps, core_ids=list(range(NCORES)))
    outs = [r["out"] for r in res.results]
    return np.ascontiguousarray(np.concatenate(outs, axis=0), dtype=np.float32)



# revision 4
# speedup vs baseline: 1.1362x; 1.1362x over previous
"""Trainium2 Bass kernel for nn_CrossAttentionBlock (hybrid PE/DVE reduce).

Reference computation (B=16384, C=1024, D=128):
    g_x     = x0 @ g_w.T + g_b          # [B, D]
    theta_x = x1 @ theta_w.T + theta_b  # [B, D]
    phi_x   = x1 @ phi_w.T + phi_b      # [B, D]
    f[b,i,j] = phi_x[b,i] * theta_x[b,j]
    attn = softmax(f, axis=-1)
    y[b,i] = sum_j attn[b,i,j] * g_x[b,j]
    out = y @ W_w.T + W_b + x0          # [B, C]

Unnormalized form on-chip (no max-subtraction needed: |f| <= ~40):
    num[i] = sum_j g[b,j] * e^{theta_j phi_i}   den[i] = sum_j e^{...}
    y[b,i] = num[i] / den[i]

Sharding: pure data parallel over batch across 8 cores (2048 rows/core).

The core is power-throttled (~50% util cap on the tensor engine), so the
attention reduce is SPLIT between the PE and the DVE by 64-row quarter:
  PE quarters (even): gen -> f_T [j,(b,i)]; reduce via per-row matmuls
      (lhsT = E_T_b, rhs = [g|1] from g1) into nd PSUM [i, 2b].
  DVE quarters (odd): gen -> f [i,(b,j)] (realign roles swapped); g is
      flattened b-major to partitions {0,32,64,96} and replicated via
      stream_shuffle; num/den via batched tensor_reduce(axis=X) in bf16.
Both halves meet in per-group divides -> y_T [i, b] -> final matmul.
"""

import os
from contextlib import ExitStack

import numpy as np

import concourse.bass as bass
import concourse.tile as tile
from concourse import bacc
from concourse import mybir

F32 = mybir.dt.float32
F16 = mybir.dt.float16
BF16 = mybir.dt.bfloat16

NCORES = 8
B, C, D = 16384, 1024, 128
KC = C // 128  # 8 contraction chunks for the projections

# batch rows per f/E tile in the attention phase (1024 fp32 = 2 PSUM banks)
FTILE = 8

# quarter q (64 rows) reduces on DVE iff q % K_DVE_MOD == K_DVE_REM
K_DVE_MOD = int(os.environ.get("K_DVE_MOD", "2"))
K_DVE_REM = int(os.environ.get("K_DVE_REM", "1"))


def build_bass(bc: int, reps: int = 1):
    """Build the per-core bass program for a batch slice of `bc` rows.

    reps>1 repeats the whole pipeline (for (T_R - T_1)/(R-1) timing)."""
    ng = bc // 128  # groups of 128 rows
    nq = max(1, bc // 512)  # 512-row groups for the g projection
    qsz = min(bc, 512)
    n_ftiles = (bc + FTILE - 1) // FTILE

    def is_dve_q(q):
        return K_DVE_MOD > 0 and q % K_DVE_MOD == K_DVE_REM

    nc = bacc.Bacc(trn_type="TRN2")

    x1t = nc.dram_tensor("x1t", [C, bc], F16, kind="ExternalInput")
    x0t = nc.dram_tensor("x0t", [C, bc], F16, kind="ExternalInput")
    x0r = nc.dram_tensor("x0r", [bc, C], F16, kind="ExternalInput")
    wc = nc.dram_tensor("wc", [C, 2 * D], F16, kind="ExternalInput")
    gwt = nc.dram_tensor("gwt", [C, D], F16, kind="ExternalInput")
    wwt = nc.dram_tensor("wwt", [D, C], BF16, kind="ExternalInput")
    btp = nc.dram_tensor("btp", [128, 3 * D], F32, kind="ExternalInput")
    gb = nc.dram_tensor("gb", [D, 1], F32, kind="ExternalInput")
    out = nc.dram_tensor("out", [bc, C], F32, kind="ExternalOutput")

    with nc.allow_low_precision("softmax num/den accumulated in bf16 on the "
                                "DVE; far inside the 2e-2 gate"), \
         tile.TileContext(nc) as tc, ExitStack() as ctx:
        singles = ctx.enter_context(tc.tile_pool(name="singles", bufs=1))

        # ---- static weights / constants in SBUF ----
        wc_sb = singles.tile([128, KC, 2 * D], F16)  # [c-part, chunk, 256]
        nc.sync.dma_start(wc_sb, wc[:, :].rearrange("(k p) d -> p k d", p=128))
        gwt_sb = singles.tile([128, KC, D], F16)
        nc.sync.dma_start(gwt_sb, gwt[:, :].rearrange("(k p) d -> p k d", p=128))
        wwt_sb = singles.tile([128, C], BF16)
        nc.sync.dma_start(wwt_sb, wwt[:, :])
        btp_sb = singles.tile([128, 3 * D], F32)
        nc.sync.dma_start(btp_sb, btp[:, :])
        gb_sb = singles.tile([128, 1], F32)
        nc.sync.dma_start(gb_sb, gb[:, :])

        # persistent per-core activations
        tp16 = singles.tile([128, ng, 3 * D], F16)  # [theta|phi|g] [b-part, G, 384]
        g1 = singles.tile([128, 2 * bc], BF16)  # g_xT interleaved with ones [d, 2b]
        y16 = singles.tile([128, bc], BF16)  # y_T [d, b] bf16
        nc.vector.memset(g1, 1.0)

        # g flattened b-major on partitions {0,32,64,96} for stream_shuffle
        # broadcast (per group's DVE rows; ping-pong across groups)
        gsp = [singles.tile([128, 64 * D], F16, name=f"gsp{i}") for i in range(2)]

        from contextlib import nullcontext
        rep_ctx = tc.For_i(0, reps, 1) if reps > 1 else nullcontext()
        with rep_ctx:
            # ===== P1 interleaved into P2: per-group projections =====
            with (
                tc.tile_pool(name="xin", bufs=4) as xin,
                tc.tile_pool(name="xg", bufs=2) as xg,
                tc.tile_pool(name="projpsum", bufs=1, space="PSUM") as projpsum,
                tc.tile_pool(name="fpsum", bufs=2, space="PSUM") as fpsum,
                tc.tile_pool(name="ndpsum", bufs=1, space="PSUM") as ndpsum,
                tc.tile_pool(name="opsum", bufs=2, space="PSUM") as opsum,
                tc.tile_pool(name="epool", bufs=6) as epool,
                tc.tile_pool(name="egpool", bufs=2) as egpool,
                tc.tile_pool(name="gbc", bufs=3) as gbc,
                tc.tile_pool(name="ndg", bufs=4) as ndg,
                tc.tile_pool(name="ndsb", bufs=2) as ndsb,
                tc.tile_pool(name="rec", bufs=2) as rec,
                tc.tile_pool(name="resid", bufs=8) as resid,
                tc.tile_pool(name="osb", bufs=3) as osb,
            ):
                g1v = g1.rearrange("p (b two) -> p b two", two=2)

                x1_tiles = [None] * ng
                x0_tiles = [None] * nq

                def emit_x1_dma(G):
                    x1_tiles[G] = xin.tile([128, KC, 128], F16, tag="xin", name="xint")
                    nc.sync.dma_start(
                        x1_tiles[G],
                        x1t[:, G * 128 : (G + 1) * 128].rearrange(
                            "(k p) b -> p k b", p=128
                        ),
                    )

                def emit_x0_dma(q):
                    x0_tiles[q] = xg.tile([128, KC, qsz], F16, tag="xg", name="xgt")
                    nc.sync.dma_start(
                        x0_tiles[q],
                        x0t[:, q * qsz : (q + 1) * qsz].rearrange(
                            "(k p) b -> p k b", p=128
                        ),
                    )

                def emit_proj_tp(G):
                    # theta/phi [b, 256] and g [b, 128] for one 128-row group
                    pt = projpsum.tile([128, 512], F32, tag="pp", name="ppt")
                    xt = x1_tiles[G]
                    for k in range(KC):
                        nc.tensor.matmul(
                            pt[:, : 2 * D], lhsT=xt[:, k, :], rhs=wc_sb[:, k, :],
                            start=(k == 0), stop=(k == KC - 1),
                        )
                    q, off = divmod(G * 128, qsz)
                    xt0 = x0_tiles[q]
                    for k in range(KC):
                        nc.tensor.matmul(
                            pt[:, 2 * D : 3 * D],
                            lhsT=xt0[:, k, off : off + 128], rhs=gwt_sb[:, k, :],
                            start=(k == 0), stop=(k == KC - 1),
                        )
                    nc.vector.tensor_add(tp16[:, G, :], pt[:, : 3 * D], btp_sb)
                    # flatten this group's DVE-quarter g rows (b-local 64:128)
                    # b-major into partitions {0,32,64,96} of gsp[G%2]
                    if K_DVE_MOD > 0:
                        src = tp16[:, G, 2 * D : 3 * D][64:128, :]
                        for rpl in range(4):
                            dst = gsp[G % 2][32 * rpl : 32 * rpl + 1, :].rearrange(
                                "o (b j) -> o b j", j=D
                            )
                            nc.sync.dma_start(dst, src)

                def emit_proj_g(q):
                    gp = projpsum.tile([128, 512], F32, tag="pp", name="gpt")
                    gp = gp[:, :qsz]
                    xt = x0_tiles[q]
                    for k in range(KC):
                        nc.tensor.matmul(
                            gp, lhsT=gwt_sb[:, k, :], rhs=xt[:, k, :],
                            start=(k == 0), stop=(k == KC - 1),
                        )
                    nc.vector.tensor_scalar_add(
                        g1v[:, q * qsz : (q + 1) * qsz, 0], gp, gb_sb
                    )

                # slot s -> (group G, b-local): natural order. The quad rows are
                # realigned onto partitions 0-3 (b mod 4 -> partition): the lhsT
                # operand compact in thbuf, the rhs operand block-diagonal in the
                # zero-padded phbuf, so one K=4 matmul computes 4 outer products.
                # PE quarters: lhsT=theta -> f_T [j,(b,i)]; DVE quarters:
                # lhsT=phi -> f [i,(b,j)].
                QB = 4  # batch rows per outer matmul
                QROWS = 64  # b-rows per realigned buffer quarter
                nquart = (bc + QROWS - 1) // QROWS
                QT = QROWS // QB  # quads per quarter

                f_tiles = [None] * n_ftiles
                e_tiles = [None] * n_ftiles
                nd_tiles = [None] * ng
                ndg_tiles = [None] * ng
                xr_tiles = [None] * ng

                gpq = max(1, qsz // 128)  # groups per g-projection block

                # ping-pong persistent realign buffers (zeros are static)
                thbuf = [
                    singles.tile([128, QT * D], F16, name=f"thbuf{i}")
                    for i in range(2)
                ]
                phbuf = [
                    singles.tile([128, QT * QB * D], F16, name=f"phbuf{i}")
                    for i in range(2)
                ]
                for i in range(2):
                    nc.vector.memset(phbuf[i], 0.0)

                def emit_realign(q):
                    # rows [q*QROWS, (q+1)*QROWS): compact row (QB*t+p) to
                    # (partition p, offset t*128); block-diag row to (partition
                    # p, offset t*512 + p*128) inside the zeroed buffer.
                    Gq, blq = divmod(q * QROWS, 128)
                    tb, pb = thbuf[q % 2], phbuf[q % 2]
                    c_lo, d_lo = (0, D) if not is_dve_q(q) else (D, 0)
                    for p in range(QB):
                        src_c = tp16[:, Gq, c_lo : c_lo + D][
                            blq + p : blq + QROWS : QB, :
                        ]
                        dst_c = tb[p : p + 1, :].rearrange("o (t e) -> o t e", e=D)
                        nc.gpsimd.dma_start(dst_c, src_c)
                        src_d = tp16[:, Gq, d_lo : d_lo + D][
                            blq + p : blq + QROWS : QB, :
                        ]
                        dst_d = pb[p : p + 1, :].rearrange(
                            "o (t f) -> o t f", f=QB * D
                        )[:, :, p * D : (p + 1) * D]
                        nc.gpsimd.dma_start(dst_d, src_d)

                def emit_outers(T):
                    lo, hi = T * FTILE, min((T + 1) * FTILE, bc)
                    f_tiles[T] = fpsum.tile([128, FTILE * 128], F32, tag="f", name="ftile")
                    for s in range(lo, hi, QB):
                        G, bl = divmod(s, 128)
                        if bl == 0:
                            # prefetch input DMAs and run projections one
                            # group ahead so the PE never stalls on loads.
                            if G == 0:
                                for Gp in range(min(3, ng)):
                                    emit_x1_dma(Gp)
                                emit_x0_dma(0)
                                emit_proj_g(0)
                                for Gp in range(min(2, ng)):
                                    emit_proj_tp(Gp)
                            else:
                                if G + 2 < ng:
                                    emit_x1_dma(G + 2)
                                if G + 1 < ng:
                                    emit_proj_tp(G + 1)
                            if (G + 1) % gpq == 0 and (G + 1) // gpq < nq:
                                emit_x0_dma((G + 1) // gpq)
                            if G % gpq == 0 and G > 0:
                                emit_proj_g(G // gpq)
                            nd_tiles[G] = ndpsum.tile([128, 2 * 128], F32, tag="nd", name="ndt")
                            ndg_tiles[G] = (
                                ndg.tile([128, 64], BF16, tag="num", name="numt"),
                                ndg.tile([128, 64], BF16, tag="den", name="dent"),
                            )
                            xr_tiles[G] = resid.tile([128, C], F16, tag="xr", name="xrt")
                            nc.sync.dma_start(
                                xr_tiles[G], x0r[G * 128 : (G + 1) * 128, :]
                            )
                        q, r = divmod(s, QROWS)
                        if r == 0:
                            if q == 0:
                                emit_realign(0)
                            if q + 1 < nquart:
                                emit_realign(q + 1)
                        t = r // QB  # quad index within quarter
                        j = s - lo
                        nc.tensor.matmul(
                            f_tiles[T][:, j * 128 : (j + QB) * 128],
                            lhsT=thbuf[q % 2][0:QB, t * D : (t + 1) * D],
                            rhs=phbuf[q % 2][0:QB, t * QB * D : (t + 1) * QB * D],
                        )

                def emit_exp(T):
                    lo, hi = T * FTILE, min((T + 1) * FTILE, bc)
                    n = hi - lo
                    e_tiles[T] = epool.tile([128, FTILE * 128], BF16, tag="e", name="etile")
                    nc.scalar.activation(
                        e_tiles[T][:, : n * 128],
                        f_tiles[T][:, : n * 128],
                        mybir.ActivationFunctionType.Exp,
                    )

                def emit_reduces_pe(T):
                    lo, hi = T * FTILE, min((T + 1) * FTILE, bc)
                    for s in range(lo, hi):
                        G, bl = divmod(s, 128)
                        j = s - lo
                        nc.tensor.matmul(
                            nd_tiles[G][:, 2 * bl : 2 * bl + 2],
                            lhsT=e_tiles[T][:, j * 128 : (j + 1) * 128],
                            rhs=g1[:, 2 * (G * 128 + bl) : 2 * (G * 128 + bl) + 2],
                        )

                def emit_reduces_dve(T):
                    lo, hi = T * FTILE, min((T + 1) * FTILE, bc)
                    n = hi - lo
                    G = lo // 128
                    o = lo - G * 128 - 64  # col offset into num_g/den_g
                    num_g, den_g = ndg_tiles[G]
                    e_v = e_tiles[T][:, : n * 128].rearrange("p (b j) -> p b j", j=128)
                    nc.vector.tensor_reduce(
                        den_g[:, o : o + n], e_v,
                        mybir.AxisListType.X, mybir.AluOpType.add,
                    )
                    gb128 = gbc.tile([128, FTILE * 128], F16, tag="gbc", name="gbct")
                    nc.vector.stream_shuffle(
                        gb128[:, : n * 128],
                        gsp[G % 2][:, o * 128 : (o + n) * 128],
                        [0] * 32,
                    )
                    eg = egpool.tile([128, FTILE * 128], BF16, tag="eg", name="egt")
                    eg_v = eg[:, : n * 128].rearrange("p (b j) -> p b j", j=128)
                    gb_v = gb128[:, : n * 128].rearrange("p (b j) -> p b j", j=128)
                    nc.vector.tensor_mul(eg_v, e_v, gb_v)
                    nc.vector.tensor_reduce(
                        num_g[:, o : o + n], eg_v,
                        mybir.AxisListType.X, mybir.AluOpType.add,
                    )

                def emit_final(G):
                    ot = osb.tile([128, C], F32, tag="ot", name="ott")
                    for h in range(2):
                        op = opsum.tile([128, 512], F32, tag="op", name="opt")
                        nc.tensor.matmul(
                            op,
                            lhsT=y16[:, G * 128 : (G + 1) * 128],
                            rhs=wwt_sb[:, h * 512 : (h + 1) * 512],
                        )
                        nc.vector.tensor_add(
                            ot[:, h * 512 : (h + 1) * 512],
                            op,
                            xr_tiles[G][:, h * 512 : (h + 1) * 512],
                        )
                    nc.sync.dma_start(out[G * 128 : (G + 1) * 128, :], ot)

                def emit_divide(G):
                    # PE half (b-local 0:64) from nd PSUM [i, (b, 2)]
                    nd = ndsb.tile([128, 128], F32, tag="ndsb")
                    nc.vector.tensor_copy(nd, nd_tiles[G][:, 0:128])
                    ndv = nd.rearrange("p (b two) -> p b two", two=2)
                    r = rec.tile([128, 128], F32, tag="rec")
                    nc.vector.reciprocal(r[:, 0:64], ndv[:, :, 1])
                    nc.vector.tensor_mul(
                        y16[:, G * 128 : G * 128 + 64], ndv[:, :, 0], r[:, 0:64]
                    )
                    # DVE half (b-local 64:128) from num_g/den_g bf16
                    num_g, den_g = ndg_tiles[G]
                    nc.vector.reciprocal(r[:, 64:128], den_g)
                    nc.vector.tensor_mul(
                        y16[:, G * 128 + 64 : (G + 1) * 128], num_g, r[:, 64:128]
                    )

                # software-pipelined emission: outers(T), exp(T-1), reduces(T-2)
                for T in range(n_ftiles + 2):
                    if T < n_ftiles:
                        emit_outers(T)
                    if 1 <= T <= n_ftiles:
                        emit_exp(T - 1)
                    if T >= 2:
                        Tr = T - 2
                        if is_dve_q(Tr * FTILE // QROWS):
                            emit_reduces_dve(Tr)
                        else:
                            emit_reduces_pe(Tr)
                        # divide+final for any group fully reduced by tile Tr
                        hi = min((Tr + 1) * FTILE, bc)
                        lo = Tr * FTILE
                        for G in range(lo // 128, (hi + 127) // 128):
                            if lo < (G + 1) * 128 <= hi:
                                emit_divide(G)
                                emit_final(G)

    nc.compile()
    return nc


_BASS_CACHE = {}


def _get_bass(bc):
    if bc not in _BASS_CACHE:
        _BASS_CACHE[bc] = build_bass(bc)
    return _BASS_CACHE[bc]


def make_core_inputs(x0, x1, g_w, g_b, theta_w, theta_b, phi_w, phi_b, W_w, W_b,
                     bc=None, ncores=NCORES):
    """Host-side preprocessing -> list of per-core input dicts."""
    n = x0.shape[0] if bc is None else bc * ncores
    bc = n // ncores

    x0 = np.asarray(x0, np.float32)[:n]
    x1 = np.asarray(x1, np.float32)[:n]
    x1t = np.ascontiguousarray(x1.T.astype(np.float16))
    x0t = np.ascontiguousarray(x0.T.astype(np.float16))
    x0r = x0 if not np.any(W_b) else (x0 + np.asarray(W_b, np.float32)[None, :])
    x0r = np.ascontiguousarray(x0r, dtype=np.float16)

    wc = np.ascontiguousarray(
        np.concatenate([np.asarray(theta_w).T, np.asarray(phi_w).T], axis=1).astype(np.float16)
    )  # [C, 2D]
    gwt = np.ascontiguousarray(np.asarray(g_w).T.astype(np.float16))  # [C, D]
    import ml_dtypes
    wwt = np.ascontiguousarray(np.asarray(W_w).T.astype(ml_dtypes.bfloat16))  # [D, C]
    btp = np.ascontiguousarray(
        np.tile(
            np.concatenate(
                [np.asarray(theta_b), np.asarray(phi_b), np.asarray(g_b)]
            )[None, :],
            (128, 1),
        ).astype(np.float32)
    )
    gbc = np.ascontiguousarray(np.asarray(g_b, np.float32).reshape(D, 1))

    in_maps = []
    for c in range(ncores):
        sl = slice(c * bc, (c + 1) * bc)
        in_maps.append(
            {
                "x1t": np.ascontiguousarray(x1t[:, sl]),
                "x0t": np.ascontiguousarray(x0t[:, sl]),
                "x0r": np.ascontiguousarray(x0r[sl]),
                "wc": wc,
                "gwt": gwt,
                "wwt": wwt,
                "btp": btp,
                "gb": gbc,
            }
        )
    return in_maps, bc


def kernel(x0, x1, g_w, g_b, theta_w, theta_b, phi_w, phi_b, W_w, W_b):
    from concourse.bass_utils import run_bass_kernel_spmd

    in_maps, bc = make_core_inputs(
        x0, x1, g_w, g_b, theta_w, theta_b, phi_w, phi_b, W_w, W_b
    )
    nc = _get_bass(bc)
    res = run_bass_kernel_spmd(nc, in_maps, core_ids=list(range(NCORES)))
    outs = [r["out"] for r in res.results]
    return np.ascontiguousarray(np.concatenate(outs, axis=0), dtype=np.float32)


# revision 14
# speedup vs baseline: 1.3530x; 1.1908x over previous
"""Trainium2 Bass kernel for nn_CrossAttentionBlock.

Reference computation (B=16384, C=1024, D=128):
    g_x     = x0 @ g_w.T + g_b          # [B, D]
    theta_x = x1 @ theta_w.T + theta_b  # [B, D]
    phi_x   = x1 @ phi_w.T + phi_b      # [B, D]
    f[b,i,j] = phi_x[b,i] * theta_x[b,j]
    attn = softmax(f, axis=-1)
    y[b,i] = sum_j attn[b,i,j] * g_x[b,j]
    out = y @ W_w.T + W_b + x0          # [B, C]

Unnormalized form used on-chip (no max-subtraction needed: |f| <= ~40, exp
fits fp32/bf16 comfortably):
    E_T[j,i] = exp(theta[b,j] * phi[b,i])        (per b, j on partitions)
    num[i] = sum_j g[b,j] * E_T[j,i]   den[i] = sum_j E_T[j,i]
    y[b,i] = num[i] / den[i]

Sharding: pure data parallel over batch across 8 cores (2048 rows/core).

Per-core pipeline:
  P1: theta/phi projections -> [b,d] fp16 tiles; g projection -> g_xT [d,b]
      interleaved with ones into g1 [d, 2b] bf16.
  P2: per-b rank-1 outer-product matmuls (K=1, lhsT=theta-row, rhs=phi-row,
      4-way row-tiled at partitions {0,32,64,96} via a realignment DMA) write
      f_T [j,i] into PSUM; grouped ACT exp (PSUM->SBUF, bf16) produces E_T;
      per-b reduce matmuls (lhsT=E_T_b, rhs=[g|1]) accumulate num/den in
      PSUM; DVE reciprocal+mul produce y_T [d,b] bf16.
  P3: final matmul (lhsT=y_T group, rhs=W_w.T, N=1024) + residual add + DMA.
"""

import os
from contextlib import ExitStack

import numpy as np

import concourse.bass as bass
import concourse.tile as tile
from concourse import bacc
from concourse import mybir

F32 = mybir.dt.float32
F16 = mybir.dt.float16
BF16 = mybir.dt.bfloat16

NCORES = 8
B, C, D = 16384, 1024, 128
KC = C // 128  # 8 contraction chunks for the projections

# batch rows per f/E tile in the attention phase (1024 fp32 = 2 PSUM banks)
FTILE = 8


def build_bass(bc: int, reps: int = 1):
    """Build the per-core bass program for a batch slice of `bc` rows.

    reps>1 repeats the whole pipeline (for (T_R - T_1)/(R-1) timing)."""
    ng = bc // 128  # groups of 128 rows
    nq = max(1, bc // 512)  # 512-row groups for the g projection
    qsz = min(bc, 512)
    n_ftiles = (bc + FTILE - 1) // FTILE

    nc = bacc.Bacc(trn_type="TRN2")

    x1t = nc.dram_tensor("x1t", [C, bc], F16, kind="ExternalInput")
    x0t = nc.dram_tensor("x0t", [C, bc], F16, kind="ExternalInput")
    x0r = nc.dram_tensor("x0r", [bc, C], F16, kind="ExternalInput")
    wc = nc.dram_tensor("wc", [C, 2 * D], F16, kind="ExternalInput")
    gwt = nc.dram_tensor("gwt", [C, D], F16, kind="ExternalInput")
    wwt = nc.dram_tensor("wwt", [D, C], BF16, kind="ExternalInput")
    btp = nc.dram_tensor("btp", [128, 2 * D], F32, kind="ExternalInput")
    gb = nc.dram_tensor("gb", [D, 1], F32, kind="ExternalInput")
    out = nc.dram_tensor("out", [bc, C], F32, kind="ExternalOutput")

    with tile.TileContext(nc) as tc, ExitStack() as ctx:
        singles = ctx.enter_context(tc.tile_pool(name="singles", bufs=1))

        # ---- static weights / constants in SBUF ----
        wc_sb = singles.tile([128, KC, 2 * D], F16)  # [c-part, chunk, 256]
        nc.sync.dma_start(wc_sb, wc[:, :].rearrange("(k p) d -> p k d", p=128))
        gwt_sb = singles.tile([128, KC, D], F16)
        nc.sync.dma_start(gwt_sb, gwt[:, :].rearrange("(k p) d -> p k d", p=128))
        wwt_sb = singles.tile([128, C], BF16)
        nc.sync.dma_start(wwt_sb, wwt[:, :])
        btp_sb = singles.tile([128, 2 * D], F32)
        nc.sync.dma_start(btp_sb, btp[:, :])
        gb_sb = singles.tile([128, 1], F32)
        nc.sync.dma_start(gb_sb, gb[:, :])

        # persistent per-core activations
        tp16 = singles.tile([128, ng, 2 * D], F16)  # [theta|phi] fp16, [b-part, G, 256]
        g1 = singles.tile([128, 2 * bc], BF16)  # g_xT interleaved with ones [d, 2b]
        y16 = singles.tile([128, bc], BF16)  # y_T [d, b] bf16
        nc.vector.memset(g1, 1.0)

        from contextlib import nullcontext
        rep_ctx = tc.For_i(0, reps, 1) if reps > 1 else nullcontext()
        with rep_ctx:
            # ===== P1 interleaved into P2: per-group projections =====
            with (
                tc.tile_pool(name="xin", bufs=3) as xin,
                tc.tile_pool(name="xg", bufs=2) as xg,
                tc.tile_pool(name="projpsum", bufs=1, space="PSUM") as projpsum,
                tc.tile_pool(name="fpsum", bufs=2, space="PSUM") as fpsum,
                tc.tile_pool(name="ndpsum", bufs=1, space="PSUM") as ndpsum,
                tc.tile_pool(name="opsum", bufs=2, space="PSUM") as opsum,
                tc.tile_pool(name="epool", bufs=3) as epool,
                tc.tile_pool(name="ndsb", bufs=2) as ndsb,
                tc.tile_pool(name="rec", bufs=2) as rec,
                tc.tile_pool(name="resid", bufs=6) as resid,
                tc.tile_pool(name="osb", bufs=3) as osb,
            ):
                g1v = g1.rearrange("p (b two) -> p b two", two=2)

                x1_tiles = [None] * ng
                x0_tiles = [None] * nq

                def emit_x1_dma(G):
                    x1_tiles[G] = xin.tile([128, KC, 128], F16, tag="xin", name="xint")
                    nc.sync.dma_start(
                        x1_tiles[G],
                        x1t[:, G * 128 : (G + 1) * 128].rearrange(
                            "(k p) b -> p k b", p=128
                        ),
                    )

                def emit_x0_dma(q):
                    x0_tiles[q] = xg.tile([128, KC, qsz], F16, tag="xg", name="xgt")
                    nc.sync.dma_start(
                        x0_tiles[q],
                        x0t[:, q * qsz : (q + 1) * qsz].rearrange(
                            "(k p) b -> p k b", p=128
                        ),
                    )

                def emit_proj_tp(G):
                    # theta/phi projection for one 128-row group
                    pt = projpsum.tile([128, 512], F32, tag="pp", name="ppt")
                    xt = x1_tiles[G]
                    for k in range(KC):
                        nc.tensor.matmul(
                            pt[:, : 2 * D], lhsT=xt[:, k, :], rhs=wc_sb[:, k, :],
                            start=(k == 0), stop=(k == KC - 1),
                        )
                    nc.vector.tensor_add(tp16[:, G, :], pt[:, : 2 * D], btp_sb)

                def emit_proj_g(q):
                    gp = projpsum.tile([128, 512], F32, tag="pp", name="gpt")
                    gp = gp[:, :qsz]
                    xt = x0_tiles[q]
                    for k in range(KC):
                        nc.tensor.matmul(
                            gp, lhsT=gwt_sb[:, k, :], rhs=xt[:, k, :],
                            start=(k == 0), stop=(k == KC - 1),
                        )
                    nc.vector.tensor_scalar_add(
                        g1v[:, q * qsz : (q + 1) * qsz, 0], gp, gb_sb
                    )

                # slot s -> (group G, b-local): natural order. Theta/phi rows
                # are realigned onto partitions 0-3 (b mod 4 -> partition), with
                # phi embedded block-diagonally in a zero-padded buffer, so one
                # K=4 matmul at tile_position (0,0) computes 4 outer products.
                # (Concurrent row-tiled positions crash the exec unit on this HW.)
                QB = 4  # batch rows per outer matmul
                QROWS = 64  # b-rows per realigned buffer quarter
                nquart = (bc + QROWS - 1) // QROWS
                QT = QROWS // QB  # quads per quarter

                f_tiles = [None] * n_ftiles
                e_tiles = [None] * n_ftiles
                nd_tiles = [None] * ng
                xr_tiles = [None] * ng

                gpq = max(1, qsz // 128)  # groups per g-projection block

                # ping-pong persistent realign buffers (zeros are static)
                thbuf = [
                    singles.tile([128, QT * D], F16, name=f"thbuf{i}")
                    for i in range(2)
                ]
                phbuf = [
                    singles.tile([128, QT * QB * D], F16, name=f"phbuf{i}")
                    for i in range(2)
                ]
                for i in range(2):
                    nc.vector.memset(phbuf[i], 0.0)

                def emit_realign(q):
                    # rows [q*QROWS, (q+1)*QROWS): theta row (QB*t+p) to
                    # (partition p, offset t*128); phi row to (partition p,
                    # offset t*512 + p*128) inside the zeroed buffer.
                    Gq, blq = divmod(q * QROWS, 128)
                    tb, pb = thbuf[q % 2], phbuf[q % 2]
                    for p in range(QB):
                        src_t = tp16[:, Gq, 0:D][blq + p : blq + QROWS : QB, :]
                        dst_t = tb[p : p + 1, :].rearrange("o (t e) -> o t e", e=D)
                        nc.gpsimd.dma_start(dst_t, src_t)
                        src_p = tp16[:, Gq, D : 2 * D][blq + p : blq + QROWS : QB, :]
                        dst_p = pb[p : p + 1, :].rearrange(
                            "o (t f) -> o t f", f=QB * D
                        )[:, :, p * D : (p + 1) * D]
                        nc.gpsimd.dma_start(dst_p, src_p)

                def emit_outers(T):
                    lo, hi = T * FTILE, min((T + 1) * FTILE, bc)
                    f_tiles[T] = fpsum.tile([128, FTILE * 128], F32, tag="f", name="ftile")
                    for s in range(lo, hi, QB):
                        G, bl = divmod(s, 128)
                        if bl == 0:
                            # prefetch input DMAs and run projections one
                            # group ahead so the PE never stalls on loads.
                            if G == 0:
                                for Gp in range(min(3, ng)):
                                    emit_x1_dma(Gp)
                                emit_x0_dma(0)
                                for Gp in range(min(2, ng)):
                                    emit_proj_tp(Gp)
                            else:
                                if G + 2 < ng:
                                    emit_x1_dma(G + 2)
                                if G + 1 < ng:
                                    emit_proj_tp(G + 1)
                            if (G + 1) % gpq == 0 and (G + 1) // gpq < nq:
                                emit_x0_dma((G + 1) // gpq)
                            if G % gpq == 0 and G > 0:
                                emit_proj_g(G // gpq)
                            nd_tiles[G] = ndpsum.tile([128, 2 * 128], F32, tag="nd", name="ndt")
                            xr_tiles[G] = resid.tile([128, C], F16, tag="xr", name="xrt")
                            nc.sync.dma_start(
                                xr_tiles[G], x0r[G * 128 : (G + 1) * 128, :]
                            )
                        q, r = divmod(s, QROWS)
                        if r == 0:
                            if q == 0:
                                emit_realign(0)
                            if q + 1 < nquart:
                                emit_realign(q + 1)
                        t = r // QB  # quad index within quarter
                        j = s - lo
                        nc.tensor.matmul(
                            f_tiles[T][:, j * 128 : (j + QB) * 128],
                            lhsT=thbuf[q % 2][0:QB, t * D : (t + 1) * D],
                            rhs=phbuf[q % 2][0:QB, t * QB * D : (t + 1) * QB * D],
                        )

                def emit_exp(T):
                    lo, hi = T * FTILE, min((T + 1) * FTILE, bc)
                    n = hi - lo
                    e_tiles[T] = epool.tile([128, FTILE * 128], BF16, tag="e", name="etile")
                    nc.scalar.activation(
                        e_tiles[T][:, : n * 128],
                        f_tiles[T][:, : n * 128],
                        mybir.ActivationFunctionType.Exp,
                    )
                    if os.environ.get("K_DOUBLE_EXP"):
                        e2 = epool.tile([128, FTILE * 128], BF16, tag="e2", name="e2tile")
                        nc.scalar.activation(
                            e2[:, : n * 128],
                            f_tiles[T][:, : n * 128],
                            mybir.ActivationFunctionType.Exp,
                        )

                def emit_reduces(T):
                    lo, hi = T * FTILE, min((T + 1) * FTILE, bc)
                    nrep = 2 if os.environ.get("K_DOUBLE_REDUCE") else 1
                    for s in range(lo, hi):
                        G, bl = divmod(s, 128)
                        j = s - lo
                        for _ in range(nrep):
                            nc.tensor.matmul(
                                nd_tiles[G][:, 2 * bl : 2 * bl + 2],
                                lhsT=e_tiles[T][:, j * 128 : (j + 1) * 128],
                                rhs=g1[:, 2 * (G * 128 + bl) : 2 * (G * 128 + bl) + 2],
                            )

                def emit_final(G):
                    ot = osb.tile([128, C], F32, tag="ot", name="ott")
                    for h in range(2):
                        op = opsum.tile([128, 512], F32, tag="op", name="opt")
                        nc.tensor.matmul(
                            op,
                            lhsT=y16[:, G * 128 : (G + 1) * 128],
                            rhs=wwt_sb[:, h * 512 : (h + 1) * 512],
                        )
                        nc.vector.tensor_add(
                            ot[:, h * 512 : (h + 1) * 512],
                            op,
                            xr_tiles[G][:, h * 512 : (h + 1) * 512],
                        )
                    nc.sync.dma_start(out[G * 128 : (G + 1) * 128, :], ot)

                def emit_divide(G):
                    nd = ndsb.tile([128, 256], F32, tag="ndsb")
                    nc.vector.tensor_copy(nd, nd_tiles[G])
                    ndv = nd.rearrange("p (b two) -> p b two", two=2)
                    r = rec.tile([128, 128], F32, tag="rec")
                    nc.vector.reciprocal(r, ndv[:, :, 1])
                    nc.vector.tensor_mul(
                        y16[:, G * 128 : (G + 1) * 128], ndv[:, :, 0], r
                    )

                # software-pipelined emission: outers(T), exp(T-1), reduces(T-2)
                for T in range(n_ftiles + 2):
                    if T < n_ftiles:
                        emit_outers(T)
                    if T == 1:
                        emit_proj_g(0)  # x0 DMA was issued at T=0; MMs here
                    if 1 <= T <= n_ftiles:
                        emit_exp(T - 1)
                    if T >= 2:
                        Tr = T - 2
                        emit_reduces(Tr)
                        # divide+final for any group fully reduced by tile Tr
                        hi = min((Tr + 1) * FTILE, bc)
                        lo = Tr * FTILE
                        for G in range(lo // 128, (hi + 127) // 128):
                            if lo < (G + 1) * 128 <= hi:
                                emit_divide(G)
                                emit_final(G)
                        if hi == bc and bc % 128 != 0:
                            emit_divide(bc // 128)
                            emit_final(bc // 128)

    nc.compile()
    return nc


_BASS_CACHE = {}


def _get_bass(bc):
    if bc not in _BASS_CACHE:
        _BASS_CACHE[bc] = build_bass(bc)
    return _BASS_CACHE[bc]


def make_core_inputs(x0, x1, g_w, g_b, theta_w, theta_b, phi_w, phi_b, W_w, W_b,
                     bc=None, ncores=NCORES):
    """Host-side preprocessing -> list of per-core input dicts."""
    n = x0.shape[0] if bc is None else bc * ncores
    bc = n // ncores

    x0 = np.asarray(x0, np.float32)[:n]
    x1 = np.asarray(x1, np.float32)[:n]
    x1t = np.ascontiguousarray(x1.T.astype(np.float16))
    x0t = np.ascontiguousarray(x0.T.astype(np.float16))
    x0r = x0 if not np.any(W_b) else (x0 + np.asarray(W_b, np.float32)[None, :])
    x0r = np.ascontiguousarray(x0r, dtype=np.float16)

    wc = np.ascontiguousarray(
        np.concatenate([np.asarray(theta_w).T, np.asarray(phi_w).T], axis=1).astype(np.float16)
    )  # [C, 2D]
    gwt = np.ascontiguousarray(np.asarray(g_w).T.astype(np.float16))  # [C, D]
    import ml_dtypes
    wwt = np.ascontiguousarray(np.asarray(W_w).T.astype(ml_dtypes.bfloat16))  # [D, C]
    btp = np.ascontiguousarray(
        np.tile(np.concatenate([np.asarray(theta_b), np.asarray(phi_b)])[None, :], (128, 1)).astype(np.float32)
    )
    gbc = np.ascontiguousarray(np.asarray(g_b, np.float32).reshape(D, 1))

    in_maps = []
    for c in range(ncores):
        sl = slice(c * bc, (c + 1) * bc)
        in_maps.append(
            {
                "x1t": np.ascontiguousarray(x1t[:, sl]),
                "x0t": np.ascontiguousarray(x0t[:, sl]),
                "x0r": np.ascontiguousarray(x0r[sl]),
                "wc": wc,
                "gwt": gwt,
                "wwt": wwt,
                "btp": btp,
                "gb": gbc,
            }
        )
    return in_maps, bc


def kernel(x0, x1, g_w, g_b, theta_w, theta_b, phi_w, phi_b, W_w, W_b):
    from concourse.bass_utils import run_bass_kernel_spmd

    in_maps, bc = make_core_inputs(
        x0, x1, g_w, g_b, theta_w, theta_b, phi_w, phi_b, W_w, W_b
    )
    nc = _get_bass(bc)
    res = run_bass_kernel_spmd(nc, in_maps, core_ids=list(range(NCORES)))
    outs = [r["out"] for r in res.results]
    return np.ascontiguousarray(np.concatenate(outs, axis=0), dtype=np.float32)

